# revision 25
# baseline (speedup 1.0000x reference)
"""Fused RoBERTa layer (attention + FFN, LoRA merged) on 8 Trainium2 cores.

Sharding: pure data-parallel over batch (16 batches -> 2 per core), no
collectives. LoRA low-rank updates are merged into the base weight matrices
on the host (x@W + (x@A)@B == x@(W + A@B) exactly in linear algebra), and the
1/sqrt(head_dim) score scale is folded into W_q/b_q.

Device dataflow per core (T=1024 tokens = 2 batches):
  - x^T (feature-major, bf16) resident in SBUF
  - q^T,k^T feature-major; V token-major with a ones-column interleaved per
    head ([k, 65] per head) so the attention-value matmul also produces the
    softmax denominator row for free
  - scores computed TRANSPOSED [k, q] so (a) the attention mask is a
    per-partition bias on the ScalarE Exp activation, and (b) exp'd scores
    are directly the rhs of the AV matmul -- no transposes in attention
  - o comes out feature-major -> O-proj directly; +x residual; LN1
  - FFN up produces F-major tiles -> Gelu evict is directly the down-proj
    operand; down + residual; LN2 -> out
All matmul operands bf16, accumulation fp32 (PSUM), softmax/LN math fp32.

SBUF is managed as a single "slab" pool of [128, 1040]-bf16 slots; later
phases allocate tiles with the tags of dead earlier tensors, so the Tile
framework recycles addresses with automatic WAR dependencies (pool closes
must be LIFO, which phase lifetimes here are not).
"""

import sys

sys.path.insert(0, "/opt/trn_rl_repo")

import numpy as np
import ml_dtypes

import concourse.bacc as bacc
import concourse.bass as bass
import concourse.tile as tile
from concourse import mybir
from concourse.bass_utils import run_bass_kernel_spmd
from concourse.masks import make_identity

BF16 = mybir.dt.bfloat16
F32 = mybir.dt.float32
NP_BF16 = np.dtype(ml_dtypes.bfloat16)

# Problem shape (hardcoded per spec)
B, S, D, H, HD, F = 16, 512, 1024, 16, 64, 4096
N_CORES = 8
TB = B // N_CORES          # batches per core
T = TB * S                 # tokens per core

MM_N = 512                 # matmul moving free dim / PSUM bank width (fp32)
P = 128


def _ceil_div(a, b):
    return (a + b - 1) // b


def build_program(cfg):
    """Build the SPMD bass program. cfg is a dict of gate flags + sizes."""
    D_, F_, T_, TB_, H_, HD_ = (cfg["D"], cfg["F"], cfg["T"], cfg["TB"],
                                cfg["H"], cfg["HD"])
    S_ = T_ // TB_
    KD = D_ // P               # D partition chunks
    KF = F_ // P               # F partition chunks
    TCH = T_ // P              # token chunks
    NT = _ceil_div(T_, MM_N)   # projection N tiles over tokens
    NTW = min(MM_N, T_)        # projection N tile width
    ND = _ceil_div(D_, MM_N)   # N tiles over D
    NDW = min(MM_N, D_)
    SKC = S_ // P              # key chunks per batch
    HPC = P // HD_             # heads per 128-partition chunk
    VW = HD_ + 1               # V' per-head width (ones column appended)
    UPW = min(1024, F_)        # wup column tile width
    UPT = F_ // UPW            # wup column tiles per D-chunk

    nc = bacc.Bacc("TRN2", target_bir_lowering=False, debug=False,
                   num_devices=N_CORES)

    # ---- DRAM I/O ----
    xT_d = nc.dram_tensor("xT", [D_, T_], BF16, kind="ExternalInput")
    x32_d = nc.dram_tensor("x32", [T_, D_], F32, kind="ExternalInput")
    wq_d = nc.dram_tensor("wq", [D_, D_], BF16, kind="ExternalInput")
    wk_d = nc.dram_tensor("wk", [D_, D_], BF16, kind="ExternalInput")
    wv_d = nc.dram_tensor("wv", [D_, D_], BF16, kind="ExternalInput")
    wo_d = nc.dram_tensor("wo", [D_, D_], BF16, kind="ExternalInput")
    wup_d = nc.dram_tensor("wup", [D_, F_], BF16, kind="ExternalInput")
    wdn_d = nc.dram_tensor("wdn", [F_, D_], BF16, kind="ExternalInput")
    bq_d = nc.dram_tensor("bq", [D_], F32, kind="ExternalInput")
    bk_d = nc.dram_tensor("bk", [D_], F32, kind="ExternalInput")
    bup_d = nc.dram_tensor("bup", [F_], F32, kind="ExternalInput")
    mask_d = nc.dram_tensor("maskT", [TB_, S_], F32, kind="ExternalInput")
    # gated (only read when the host says they are non-trivial)
    bv_d = nc.dram_tensor("bv", [D_], F32, kind="ExternalInput")
    bo_d = nc.dram_tensor("bo", [D_], F32, kind="ExternalInput")
    bdn_d = nc.dram_tensor("bdn", [D_], F32, kind="ExternalInput")
    g1_d = nc.dram_tensor("g1", [D_], F32, kind="ExternalInput")
    b1_d = nc.dram_tensor("b1", [D_], F32, kind="ExternalInput")
    g2_d = nc.dram_tensor("g2", [D_], F32, kind="ExternalInput")
    b2_d = nc.dram_tensor("b2", [D_], F32, kind="ExternalInput")
    out_d = nc.dram_tensor("out", [T_, D_], F32, kind="ExternalOutput")

    with tile.TileContext(nc) as tc, \
         tc.tile_pool(name="consts", bufs=1) as consts, \
         tc.tile_pool(name="slab", bufs=1) as slab, \
         tc.tile_pool(name="ps", bufs=2, space="PSUM") as ps, \
         tc.tile_pool(name="psS", bufs=2, space="PSUM") as psS, \
         tc.tile_pool(name="psV", bufs=2, space="PSUM") as psV, \
         tc.tile_pool(name="psT", bufs=2, space="PSUM") as psT, \
         tc.tile_pool(name="work", bufs=2) as work, \
         tc.tile_pool(name="x32p", bufs=2) as x32p, \
         tc.tile_pool(name="attnp", bufs=10) as attnp, \
         tc.tile_pool(name="attn2", bufs=3) as attn2, \
         tc.tile_pool(name="statp", bufs=4) as statp, \
         tc.tile_pool(name="outp", bufs=2) as outp, \
         tc.tile_pool(name="dramp", bufs=6, space="DRAM") as dramp:

        dma = nc.sync

        def slot(tag, width=None, dtype=BF16):
            # one [128, VW*HPC*... ] slab slot; width defaults to max layout
            w = (D_ // HD_) * VW if width is None else width
            return slab.tile([P, w], dtype, tag=tag, name=f"t_{tag}")

        # ---- constants ----
        eps_t = consts.tile([P, 1], F32)
        nc.vector.memset(eps_t, 1e-5)
        ident = consts.tile([P, P], BF16)
        make_identity(nc, ident)
        # per-partition bias layouts: col m <- bias[m*128:(m+1)*128]
        bq_sb = consts.tile([P, KD], F32)
        dma.dma_start(out=bq_sb, in_=bq_d.ap().rearrange("(m p) -> p m", p=P))
        bk_sb = consts.tile([P, KD], F32)
        dma.dma_start(out=bk_sb, in_=bk_d.ap().rearrange("(m p) -> p m", p=P))
        bup_sb = consts.tile([P, KF], F32)
        dma.dma_start(out=bup_sb, in_=bup_d.ap().rearrange("(m p) -> p m", p=P))
        # mask as per-partition bias: col (b*SKC+kc) <- mask[b, kc*128:+128]
        mask_sb = consts.tile([P, TB_ * SKC], F32)
        dma.dma_start(out=mask_sb,
                      in_=mask_d.ap().rearrange("b (kc p) -> p (b kc)", p=P))

        def bcast_row(dram_vec, n):
            # [n] fp32 -> [P, n] tile broadcast across partitions
            t = consts.tile([P, n], F32, name=f"bc_{dram_vec.name}")
            dma.dma_start(out=t,
                          in_=dram_vec.ap().unsqueeze(0).to_broadcast([P, n]))
            return t

        bv_bc = bcast_row(bv_d, D_) if cfg["has_bv"] else None
        bo_bc = bcast_row(bo_d, D_) if cfg["has_bo"] else None
        bdn_bc = bcast_row(bdn_d, D_) if cfg["has_bdn"] else None
        g1_bc = bcast_row(g1_d, D_) if cfg["has_n1"] else None
        b1_bc = bcast_row(b1_d, D_) if cfg["has_n1"] else None
        g2_bc = bcast_row(g2_d, D_) if cfg["has_n2"] else None
        b2_bc = bcast_row(b2_d, D_) if cfg["has_n2"] else None

        # ---- load x^T and QKV weights ----
        xT_sb = [slot(f"xT{c}", T_) for c in range(KD)]
        w_sb = {nm: [slot(f"w{nm}{c}", D_) for c in range(KD)]
                for nm in ("q", "k", "v")}
        for c in range(KD):
            dma.dma_start(out=xT_sb[c], in_=xT_d[c * P:(c + 1) * P, :])
            for nm, wd in (("q", wq_d), ("k", wk_d), ("v", wv_d)):
                dma.dma_start(out=w_sb[nm][c], in_=wd[c * P:(c + 1) * P, :])

        qT_sb = [slot(f"qT{c}", T_) for c in range(KD)]
        # k^T is stored twice with the other head-half zeroed, so attention
        # scores can contract over the full 128 partitions (K=64 matmuls
        # de-rate the whole PE stream on this part)
        kTe_sb = [slot(f"kTe{c}", T_) for c in range(KD)]
        kTo_sb = [slot(f"kTo{c}", T_) for c in range(KD)]
        for c in range(KD):
            nc.vector.memset(kTe_sb[c][P // 2:P, :], 0.0)
            nc.vector.memset(kTo_sb[c][0:P // 2, :], 0.0)
        Vp_sb = [slot(f"Vp{c}") for c in range(TCH)]

        # ---- QKV projections ----
        HB = P // 2
        for nm, bias in (("q", bq_sb), ("k", bk_sb)):
            for m in range(KD):
                for t2 in range(NT):
                    pt = ps.tile([P, MM_N], F32, tag="ps", name="ps_qk")
                    for kc in range(KD):
                        nc.tensor.matmul(
                            pt[:, :NTW],
                            lhsT=w_sb[nm][kc][:, m * P:(m + 1) * P],
                            rhs=xT_sb[kc][:, t2 * MM_N:t2 * MM_N + NTW],
                            start=(kc == 0), stop=(kc == KD - 1))
                    sl = slice(t2 * MM_N, t2 * MM_N + NTW)
                    if nm == "q":
                        nc.vector.tensor_scalar_add(
                            out=qT_sb[m][:, sl],
                            in0=pt[:, :NTW], scalar1=bias[:, m:m + 1])
                    else:
                        nc.vector.tensor_scalar_add(
                            out=kTe_sb[m][0:HB, sl],
                            in0=pt[0:HB, :NTW], scalar1=bias[0:HB, m:m + 1])
                        nc.vector.tensor_scalar_add(
                            out=kTo_sb[m][HB:P, sl],
                            in0=pt[HB:P, :NTW], scalar1=bias[HB:P, m:m + 1])
        # V token-major, scattered into V' layout (ones col per head)
        for tr in range(TCH):
            for n2 in range(ND):
                pt = ps.tile([P, MM_N], F32, tag="ps", name="ps_v")
                for kc in range(KD):
                    nc.tensor.matmul(
                        pt[:, :NDW],
                        lhsT=xT_sb[kc][:, tr * P:(tr + 1) * P],
                        rhs=w_sb["v"][kc][:, n2 * MM_N:n2 * MM_N + NDW],
                        start=(kc == 0), stop=(kc == KD - 1))
                hpn = NDW // HD_  # heads per N tile
                dst = Vp_sb[tr].rearrange("p (h c) -> p h c", c=VW)
                src = pt[:, :NDW].rearrange("p (h c) -> p h c", c=HD_)
                if cfg["has_bv"]:
                    tmp = work.tile([P, NDW], F32, tag="vtmp", name="vtmp")
                    nc.vector.tensor_add(
                        out=tmp, in0=pt[:, :NDW],
                        in1=bv_bc[:, n2 * MM_N:n2 * MM_N + NDW])
                    src = tmp.rearrange("p (h c) -> p h c", c=HD_)
                nc.vector.tensor_copy(
                    out=dst[:, n2 * hpn:(n2 + 1) * hpn, 0:HD_], in_=src)
            nc.vector.memset(
                Vp_sb[tr].rearrange("p (h c) -> p h c", c=VW)[:, :, HD_:VW],
                1.0)

        skip = cfg.get("skip", set())
        # ---- attention -> o^T feature-major ----
        wo_sb = []
        for c in range(KD):
            t = slot(f"xT{c}", D_)   # reuse xT slots
            dma.dma_start(out=t, in_=wo_d[c * P:(c + 1) * P, :])
            wo_sb.append(t)
        oT_sb = [slot(f"wq{c}", T_) for c in range(KD)]  # reuse wq slots

        if "attn" in skip:
            for c in range(KD):
                nc.vector.tensor_copy(out=oT_sb[c], in_=qT_sb[c])
        attn_iter = [] if "attn" in skip else [(b, h) for b in range(TB_)
                                               for h in range(H_)]
        for b, h in attn_iter:
                hc, ho = h // HPC, (h % HPC) * HD_
                # scores^T [k, q] per key chunk; exp via ScalarE (mask = bias)
                at_tiles = []
                for kc in range(SKC):
                    at = attnp.tile([P, S_], BF16, tag="attnT", name="attnT")
                    if True:
                        kTm = kTe_sb if (h % HPC) == 0 else kTo_sb
                        pt = psS.tile([P, MM_N], F32, tag="psS", name="ps_s")
                        nc.tensor.matmul(
                            pt[:, :S_],
                            lhsT=kTm[hc][:, b * S_ + kc * P:
                                         b * S_ + (kc + 1) * P],
                            rhs=qT_sb[hc][:, b * S_:(b + 1) * S_],
                            start=True, stop=True)
                        nc.scalar.activation(
                            out=at, in_=pt[:, :S_],
                            func=mybir.ActivationFunctionType.Exp,
                            bias=mask_sb[:, b * SKC + kc:b * SKC + kc + 1],
                            scale=1.0)
                    at_tiles.append(at)
                # o^T rows [HD] + denominator row, accumulated over key chunks
                pv = psV.tile([P, MM_N], F32, tag="psV", name="ps_v2")
                for kc in range(SKC):
                    nc.tensor.matmul(
                        pv[0:VW, :S_],
                        lhsT=Vp_sb[(b * S_) // P + kc][:,
                                                       h * VW:(h + 1) * VW],
                        rhs=at_tiles[kc],
                        start=(kc == 0), stop=(kc == SKC - 1))
                rs = attn2.tile([1, S_], F32, tag="rsum", name="rsum")
                nc.scalar.copy(out=rs, in_=pv[HD_:VW, :S_])
                rs_d = dramp.tile([1, S_], F32, tag="rs_d", name="rs_d")
                dma.dma_start(out=rs_d, in_=rs)
                rb = attn2.tile([HD_, S_], F32, tag="rsbc", name="rsbc")
                dma.dma_start(out=rb, in_=rs_d.to_broadcast([HD_, S_]))
                nc.vector.reciprocal_approx_fast(out=rb, in_=rb)
                nc.vector.tensor_mul(
                    out=oT_sb[hc][ho:ho + HD_, b * S_:(b + 1) * S_],
                    in0=pv[0:HD_, :S_], in1=rb)

        # ---- O projection + residual + LN1 -> x_medium (bf16) + transpose ----
        xm_bf = [slot(f"wk{c}", D_) for c in range(TCH)]   # reuse wk slots
        xmT_sb = [slot(f"wv{c}", T_) for c in range(KD)]   # reuse wv slots
        up_tags = ([f"qT{c}" for c in range(KD)]
                   + [f"kTe{c}" for c in range(KD)]
                   + [f"kTo{c}" for c in range(KD)]
                   + [f"Vp{c}" for c in range(TCH)]
                   + [f"u{c}" for c in range(max(0, KD * UPT - 3 * KD - TCH))])
        wup_sb = []
        for i in range(KD * UPT):
            t = slot(up_tags[i], UPW)
            dk, j = i // UPT, i % UPT
            dma.dma_start(out=t, in_=wup_d[dk * P:(dk + 1) * P,
                                           j * UPW:(j + 1) * UPW])
            wup_sb.append(t)

        def wup_lhsT(dk, fm):
            i = dk * UPT + (fm * P) // UPW
            o = (fm * P) % UPW
            return wup_sb[i][:, o:o + P]

        def layer_norm(src, dst, g_bc, b_bc):
            # src fp32 [P, D_]; dst [P, D_] (any dtype); per-partition stats
            bw = min(512, D_)
            nsub = _ceil_div(D_, bw)
            st = statp.tile([P, nsub, 6], F32, tag="bnst", name="bnst")
            for i in range(nsub):
                nc.vector.bn_stats(out=st[:, i, :],
                                   in_=src[:, i * bw:(i + 1) * bw])
            mv = statp.tile([P, 2], F32, tag="bnmv", name="bnmv")
            nc.vector.bn_aggr(out=mv, in_=st)
            rstd = statp.tile([P, 1], F32, tag="rstd", name="rstd")
            nc.scalar.activation(out=rstd, in_=mv[:, 1:2],
                                 func=mybir.ActivationFunctionType.Sqrt,
                                 bias=eps_t, scale=1.0)
            nc.vector.reciprocal(out=rstd, in_=rstd)
            if g_bc is None:
                nc.vector.tensor_scalar(
                    out=dst, in0=src, scalar1=mv[:, 0:1], scalar2=rstd,
                    op0=mybir.AluOpType.subtract, op1=mybir.AluOpType.mult)
            else:
                tmp = statp.tile([P, D_], F32, tag="lntmp", name="lntmp")
                nc.vector.tensor_scalar(
                    out=tmp, in0=src, scalar1=mv[:, 0:1], scalar2=rstd,
                    op0=mybir.AluOpType.subtract, op1=mybir.AluOpType.mult)
                nc.vector.tensor_mul(out=tmp, in0=tmp, in1=g_bc)
                nc.vector.tensor_add(out=dst, in0=tmp, in1=b_bc)

        for tr in range(TCH):
            xt = x32p.tile([P, D_], F32, tag="x32t", name="x32t")
            dma.dma_start(out=xt, in_=x32_d[tr * P:(tr + 1) * P, :])
            of = work.tile([P, D_], F32, tag="acc", name="of")
            for n2 in range(ND):
                pt = ps.tile([P, MM_N], F32, tag="ps", name="ps_o")
                for kc in range(KD):
                    nc.tensor.matmul(
                        pt[:, :NDW],
                        lhsT=oT_sb[kc][:, tr * P:(tr + 1) * P],
                        rhs=wo_sb[kc][:, n2 * MM_N:n2 * MM_N + NDW],
                        start=(kc == 0), stop=(kc == KD - 1))
                nc.vector.tensor_add(out=of[:, n2 * MM_N:n2 * MM_N + NDW],
                                     in0=pt[:, :NDW],
                                     in1=xt[:, n2 * MM_N:n2 * MM_N + NDW])
                if cfg["has_bo"]:
                    nc.vector.tensor_add(
                        out=of[:, n2 * MM_N:n2 * MM_N + NDW],
                        in0=of[:, n2 * MM_N:n2 * MM_N + NDW],
                        in1=bo_bc[:, n2 * MM_N:n2 * MM_N + NDW])
            if "ln" in skip:
                nc.vector.tensor_copy(out=xm_bf[tr], in_=of)
            else:
                layer_norm(of, xm_bf[tr],
                           g1_bc if cfg["has_n1"] else None,
                           b1_bc if cfg["has_n1"] else None)
            # transpose x_medium chunk -> xmT (PE transpose via identity)
            for c in range(KD):
                if "tr" in skip:
                    nc.vector.tensor_copy(
                        out=xmT_sb[c][:, tr * P:(tr + 1) * P],
                        in_=xm_bf[tr][:, c * P:(c + 1) * P])
                else:
                    pt = psT.tile([P, P], BF16, tag="psT", name="ps_t")
                    nc.tensor.transpose(pt, xm_bf[tr][:, c * P:(c + 1) * P],
                                        ident)
                    nc.scalar.copy(
                        out=xmT_sb[c][:, tr * P:(tr + 1) * P], in_=pt)

        # ---- FFN up (F-major out) + Gelu -> gT ----
        g_tags = ([f"xT{c}" for c in range(KD)] + [f"wq{c}" for c in range(KD)]
                  + [f"g{c}" for c in range(KF - 2 * KD)])
        gT_sb = [slot(g_tags[c], T_) for c in range(KF)]
        for fm in range(KF):
            for t2 in range(NT):
                pt = ps.tile([P, MM_N], F32, tag="ps", name="ps_up")
                for kc in range(KD):
                    nc.tensor.matmul(
                        pt[:, :NTW],
                        lhsT=wup_lhsT(kc, fm),
                        rhs=xmT_sb[kc][:, t2 * MM_N:t2 * MM_N + NTW],
                        start=(kc == 0), stop=(kc == KD - 1))
                nc.scalar.activation(
                    out=gT_sb[fm][:, t2 * MM_N:t2 * MM_N + NTW],
                    in_=pt[:, :NTW],
                    func=mybir.ActivationFunctionType.Gelu,
                    bias=bup_sb[:, fm:fm + 1], scale=1.0)

        # ---- FFN down + residual + LN2 -> out ----
        wdn_sb = []
        for fc in range(KF):
            t = slot(up_tags[fc], D_)   # reuse wup slots
            dma.dma_start(out=t, in_=wdn_d[fc * P:(fc + 1) * P, :])
            wdn_sb.append(t)
        for tr in range(TCH):
            dsb = work.tile([P, D_], F32, tag="acc", name="dsb")
            for n2 in range(ND):
                pt = ps.tile([P, MM_N], F32, tag="ps", name="ps_dn")
                for fc in range(KF):
                    nc.tensor.matmul(
                        pt[:, :NDW],
                        lhsT=gT_sb[fc][:, tr * P:(tr + 1) * P],
                        rhs=wdn_sb[fc][:, n2 * MM_N:n2 * MM_N + NDW],
                        start=(fc == 0), stop=(fc == KF - 1))
                nc.vector.tensor_add(
                    out=dsb[:, n2 * MM_N:n2 * MM_N + NDW],
                    in0=pt[:, :NDW],
                    in1=xm_bf[tr][:, n2 * MM_N:n2 * MM_N + NDW])
                if cfg["has_bdn"]:
                    nc.vector.tensor_add(
                        out=dsb[:, n2 * MM_N:n2 * MM_N + NDW],
                        in0=dsb[:, n2 * MM_N:n2 * MM_N + NDW],
                        in1=bdn_bc[:, n2 * MM_N:n2 * MM_N + NDW])
            ot = outp.tile([P, D_], F32, tag="ot", name="ot")
            if "ln" in skip:
                nc.vector.tensor_copy(out=ot, in_=dsb)
            else:
                layer_norm(dsb, ot,
                           g2_bc if cfg["has_n2"] else None,
                           b2_bc if cfg["has_n2"] else None)
            dma.dma_start(out=out_d[tr * P:(tr + 1) * P, :], in_=ot)

    nc.finalize()
    return nc


_PROGRAM_CACHE = {}


def _get_program(cfg_key, cfg):
    if cfg_key not in _PROGRAM_CACHE:
        _PROGRAM_CACHE[cfg_key] = build_program(cfg)
    return _PROGRAM_CACHE[cfg_key]


def make_in_maps(inputs):
    """Host-side prep: LoRA merge, scale fold, dtype conversion, sharding."""
    f32 = np.float32
    x = np.asarray(inputs["x"], f32)
    scale = 1.0 / np.sqrt(float(inputs["head_dim"]))

    def merged(w, a, b):
        return (np.asarray(w, f32)
                + np.asarray(a, f32) @ np.asarray(b, f32))

    wq = (merged(inputs["w_q"], inputs["w_q_lora_a"], inputs["w_q_lora_b"])
          * scale).astype(NP_BF16)
    wk = merged(inputs["w_k"], inputs["w_k_lora_a"],
                inputs["w_k_lora_b"]).astype(NP_BF16)
    wv = merged(inputs["w_v"], inputs["w_v_lora_a"],
                inputs["w_v_lora_b"]).astype(NP_BF16)
    wo = merged(inputs["w_o"], inputs["w_o_lora_a"],
                inputs["w_o_lora_b"]).astype(NP_BF16)
    wup = merged(inputs["w_up"], inputs["w_up_lora_a"],
                 inputs["w_up_lora_b"]).astype(NP_BF16)
    wdn = merged(inputs["w_down"], inputs["w_down_lora_a"],
                 inputs["w_down_lora_b"]).astype(NP_BF16)
    mask = np.asarray(inputs["attention_mask"], f32)  # [B,1,1,S]

    common = {
        "wq": wq, "wk": wk, "wv": wv, "wo": wo, "wup": wup, "wdn": wdn,
        "bq": (np.asarray(inputs["b_q"], f32) * scale).astype(f32),
        "bk": np.asarray(inputs["b_k"], f32),
        "bup": np.asarray(inputs["b_up"], f32),
        "bv": np.asarray(inputs["b_v"], f32),
        "bo": np.asarray(inputs["b_o"], f32),
        "bdn": np.asarray(inputs["b_down"], f32),
        "g1": np.asarray(inputs["norm_weight_1"], f32),
        "b1": np.asarray(inputs["norm_bias_1"], f32),
        "g2": np.asarray(inputs["norm_weight_2"], f32),
        "b2": np.asarray(inputs["norm_bias_2"], f32),
    }
    in_maps = []
    for i in range(N_CORES):
        xc = x[i * TB:(i + 1) * TB].reshape(T, D)
        m = dict(common)
        m["xT"] = np.ascontiguousarray(xc.T).astype(NP_BF16)
        m["x32"] = np.ascontiguousarray(xc)
        m["maskT"] = np.ascontiguousarray(mask[i * TB:(i + 1) * TB, 0, 0, :])
        in_maps.append(m)
    return in_maps


def full_cfg(inputs):
    f32 = np.float32
    return {
        "D": D, "F": F, "T": T, "TB": TB, "H": H, "HD": HD,
        "has_bv": bool(np.any(np.asarray(inputs["b_v"], f32))),
        "has_bo": bool(np.any(np.asarray(inputs["b_o"], f32))),
        "has_bdn": bool(np.any(np.asarray(inputs["b_down"], f32))),
        "has_n1": bool(np.any(np.asarray(inputs["norm_weight_1"], f32) != 1.0)
                       or np.any(np.asarray(inputs["norm_bias_1"], f32))),
        "has_n2": bool(np.any(np.asarray(inputs["norm_weight_2"], f32) != 1.0)
                       or np.any(np.asarray(inputs["norm_bias_2"], f32))),
    }


def run_on_hw(inputs, trace=False, tmpdir=None):
    cfg = full_cfg(inputs)
    cfg_key = tuple(sorted((k, v) for k, v in cfg.items()))
    nc = _get_program(cfg_key, cfg)
    in_maps = make_in_maps(inputs)
    kw = {}
    if trace:
        kw = {"trace": True, "tmpdir": tmpdir}
    res = run_bass_kernel_spmd(nc, in_maps, core_ids=list(range(N_CORES)),
                               **kw)
    out = np.empty((B, S, D), np.float32)
    for i in range(N_CORES):
        out[i * TB:(i + 1) * TB] = res.results[i]["out"].reshape(TB, S, D)
    return out, res


def kernel(**inputs):
    out, _ = run_on_hw(inputs)
    return out
